# revision 27
# baseline (speedup 1.0000x reference)
"""Trainium2 Bass kernel for nn_PixelEffectModule (histogram binning pixelation).

Reference semantics (rgb [1,3,H,W], num_bins=4, kernel_size=3, pixel_size=32):
  idx  = int32(mean(rgb, axis=(0,1)) / 256 * 4)            # per-pixel bin
  per output cell (i,j) (H/32 x W/32 grid), over the 3x3 pixel window
  centered at (32i, 32j) (zero-padded at borders):
    cnt[b]   = #pixels in window with idx == b
    sum_c[b] = sum of channel c over those pixels
  b* = argmax_b cnt[b] (first max wins)
  out color at cell = sum_c[b*] / cnt[b*], nearest-upsampled 32x in both dims.

Key facts exploited:
  - stride 32 + kernel 3 => only 3 rows/cols per 32 are ever read
  - output is constant on 32x32 blocks => generate on chip, write 6 MiB/core
  - sharding: core m handles 8 block-rows (256 output rows)

Per-core device program:
  x  [72,192] f32 : packed pixels; partition (k,u,v) = (block-row 0..7,
       win-row 0..2, win-col 0..2); free (c,j) = (channel 0..2, block-col 0..63).
       Out-of-bounds pixels filled with 1e7 (maps to no bin).
  w  [72,256] f32 : two ones-matrices; w[(k,u,v), h*128+p] = (k == 4h + p//32)
       -> matmul sums over (u,v) AND replicates each cell row over 32
       partitions so every post-stage op runs 128 lanes wide.
  y  [3,256,2048] f32 : this core's output rows.

Binning compares sum=(r+g)+b against precomputed f32 thresholds
T[b] = min f32 v with f32_div(v,3) >= 64*b, replicating the reference's
float32 rounding bit-exactly (the /256*4 steps are exact powers of two).
Argmax-with-first-tie-break is max of score = 4*cnt + (3-b), all exact
small ints in f32.
"""

import numpy as np

import concourse.bass as bass
import concourse.tile as tile
from concourse import mybir
from concourse.alu_op_type import AluOpType
from concourse.bass_utils import run_bass_kernel_spmd

F32 = mybir.dt.float32

H = W = 2048
PS = 32          # pixel_size
KS = 3           # kernel_size
NB = 4           # num_bins
GH = H // PS     # 64 block rows
GW = W // PS     # 64 block cols
NCORES = 8
KPC = GH // NCORES          # 8 block rows per core
ROWS_PC = KPC * PS          # 256 output rows per core
KUV = KPC * KS * KS         # 72 partitions of packed pixels
FILL = 1.0e7                # OOB fill; sum=3e7 matches no bin


def _bin_thresholds():
    """Thresholds on s=(r+g)+b replicating the axon-jax binning bit-exactly.

    On this backend: mean = f32(s * f32(1/3)) (mult-by-reciprocal), and the
    f32->int32 cast is round-half-even, so idx = rhe(mean/64):
      idx>=1 iff mean > 32;  idx>=2 iff mean >= 96;
      idx>=3 iff mean > 160; idx==4 (void, outside one_hot) iff mean >= 224.
    OOB-filled pixels (s ~ 3e7) land past the void threshold too.
    """
    c = np.float32(1.0) / np.float32(3.0)
    out = []
    for target, strict in ((32, True), (96, False), (160, True), (224, False)):
        t = np.float32(target)
        if strict:
            pred = lambda x: np.float32(x) * c > t
        else:
            pred = lambda x: np.float32(x) * c >= t
        v = np.float32(target * 3)
        while pred(v):
            v = np.nextafter(v, np.float32(-np.inf), dtype=np.float32)
        while not pred(v):
            v = np.nextafter(v, np.float32(np.inf), dtype=np.float32)
        out.append(float(v))
    return out


class _SplitDrainTileContext(tile.TileContext):
    """TileContext whose final Drain's sem waits are split one-per-NOP.

    The stock exit path attaches every ticked sem lane (PE, DVE, each DMA
    lane, ...) as waits on a single SP Drain; walrus rejects CTRL
    instructions with >2 sync waits. SP executes in order, so moving each
    wait onto its own preceding single-wait NoOp is equivalent.
    """

    def _drain_and_barrier(self, tick_clock, wait_clock):
        from concourse.vector_clock import ScopedClock

        nc = self.nc
        pre = [nc.sync.nop(nofuse=True) for _ in range(12)]
        drain_inst = nc.sync.drain()
        wait_clock.add_sem_waits(
            drain_inst.ins, ScopedClock({None: tick_clock.global_clock})
        )
        si = drain_inst.ins.sync_info
        if si is not None and len(si.on_wait) > 1:
            waits = list(si.on_wait)
            assert len(waits) <= len(pre), f"{len(waits)} drain waits"
            for w, n in zip(waits, pre):
                n.ins.sync_info = mybir.SyncInfo(on_wait=[w], on_update=[])
            drain_inst.ins.sync_info = mybir.SyncInfo(
                on_wait=[], on_update=list(si.on_update)
            )

        nc.all_engine_barrier()
        assert self.sems is not None
        popped = nc._tile_sem_poison_stack.pop()
        assert popped is self._sem_poison
        nc.clear_and_free_semaphores(list(self.sems.allocated().values()))
        nc.all_engine_barrier()


def _build_nc():
    nc = bass.Bass()
    # packed pixels (cols 0:192, layout (c,j)) + selector matrix (cols 192:448)
    # in ONE dram tensor / one DMA: fewer DMAHW sems keeps the final Drain
    # under the sync-wait limit
    x = nc.dram_tensor("x", [KUV, 3 * GW + 256], F32, kind="ExternalInput")
    # [row, c, w] layout (host transposes after gather): the whole 6 MiB
    # output is then a single 3-dim DMA AP -> one output DMA -> the final
    # Drain stays at 4 sem waits (PE, DVE, in-DMA, out-DMA), its HW limit
    y = nc.dram_tensor("y", [ROWS_PC, 3, W], F32, kind="ExternalOutput")

    T = _bin_thresholds()

    with _SplitDrainTileContext(nc) as tc:
        with (
            tc.tile_pool(name="pool", bufs=1) as pool,
            tc.tile_pool(name="expool", bufs=1) as expool,
            tc.tile_pool(name="psum", bufs=2, space="PSUM") as psum,
        ):
            xw = pool.tile([KUV, 3 * GW + 256], F32, tag="xw")
            nc.sync.dma_start(xw[:], x[:])
            xt = xw[:, 0:3 * GW].rearrange("p (c j) -> p c j", c=3)
            # matmul (LoadWeights) supports a single sync wait; funnel wt
            # through DVE so matmuls depend on one semaphore only
            wtc = pool.tile([KUV, 256], F32, tag="wtc")
            nc.vector.tensor_copy(wtc[:], xw[:, 3 * GW:])

            # per-pixel channel sum, replicating reference add order (r+g)+b
            sm = pool.tile([KUV, GW], F32, tag="sm")
            nc.vector.tensor_tensor(sm[:], xt[:, 0, :], xt[:, 1, :], AluOpType.add)
            nc.vector.tensor_tensor(sm[:], sm[:], xt[:, 2, :], AluOpType.add)

            # ge[b] = (sum >= T[b]), for thresholds of bins 1..3 plus void
            ge = pool.tile([KUV, 4, GW], F32, tag="ge")
            for b in range(4):
                nc.vector.tensor_scalar(
                    ge[:, b, :], sm[:], T[b], None, AluOpType.is_ge
                )

            # masked quantities M[(b,q),j]; q=0 count mask, q=1..3 channels
            M = pool.tile([KUV, NB, 4, GW], F32, tag="M")
            # eq_0 = 1 - ge1 ; eq_b = ge_b - ge_{b+1}
            nc.vector.tensor_scalar(
                M[:, 0, 0, :], ge[:, 0, :], -1.0, 1.0, AluOpType.mult, AluOpType.add
            )
            for b in range(1, 4):
                nc.vector.tensor_tensor(
                    M[:, b, 0, :], ge[:, b - 1, :], ge[:, b, :], AluOpType.subtract
                )
            for b in range(4):
                nc.vector.tensor_tensor(
                    M[:, b, 1:4, :], xt[:, :, :],
                    M[:, b, 0, :].unsqueeze(1).broadcast_to([KUV, 3, GW]),
                    AluOpType.mult,
                )

            Mflat = M[:].rearrange("p b q j -> p (b q j)")

            # on-chip 32x column expansion of both row-halves; one buffer so
            # the whole 6 MiB leaves in a single DMA
            exfull = expool.tile([128, 2, 3, W], F32, tag="ex")

            for h in range(2):
                # matmul: sums over (u,v), replicates cell-row over 32
                # partitions: out partition p -> block-row 4h + p//32
                Y = psum.tile([128, NB, 4, GW], F32, name=f"Yps{h}", tag="Yps")
                Yflat = Y[:].rearrange("p b q j -> p (b q j)")
                for b in range(NB):
                    nc.tensor.matmul(
                        Yflat[:, b * 256:(b + 1) * 256],
                        wtc[:, h * 128:(h + 1) * 128],
                        Mflat[:, b * 256:(b + 1) * 256],
                        start=True, stop=True,
                    )

                # scores: 4*cnt + (3-b)  (argmax-with-first-tie-break as max)
                S = pool.tile([128, NB, GW], F32, name=f"S{h}", tag="S", bufs=2)
                for b in range(4):
                    nc.vector.tensor_scalar(
                        S[:, b, :], Y[:, b, 0, :], 4.0, float(3 - b),
                        AluOpType.mult, AluOpType.add,
                    )
                best = pool.tile([128, GW], F32, name=f"best{h}", tag="best", bufs=2)
                nc.vector.tensor_reduce(
                    best[:], S[:].transpose([0, 2, 1]),
                    axis=mybir.AxisListType.X, op=AluOpType.max,
                )
                sel = pool.tile([128, NB, GW], F32, name=f"sel{h}", tag="sel", bufs=2)
                nc.vector.tensor_tensor(
                    sel[:], S[:],
                    best[:].unsqueeze(1).broadcast_to([128, NB, GW]),
                    AluOpType.is_equal,
                )

                # RES[q] = sum_b sel[b] * Y[b,q]  (q=0 den, q=1..3 rgb sums)
                RES = pool.tile([128, 4, GW], F32, name=f"RES{h}", tag="RES", bufs=2)
                for q in range(4):
                    prod = pool.tile(
                        [128, NB, GW], F32, name=f"prod{h}_{q}", tag="prod", bufs=2
                    )
                    nc.vector.tensor_tensor(
                        prod[:], sel[:], Y[:, :, q, :], AluOpType.mult
                    )
                    nc.vector.tensor_reduce(
                        RES[:, q, :], prod[:].transpose([0, 2, 1]),
                        axis=mybir.AxisListType.X, op=AluOpType.add,
                    )

                rcp = pool.tile([128, GW], F32, name=f"rcp{h}", tag="rcp", bufs=2)
                nc.vector.reciprocal(rcp[:], RES[:, 0, :])
                col = pool.tile([128, 3, GW], F32, name=f"col{h}", tag="col", bufs=2)
                nc.vector.tensor_tensor(
                    col[:], RES[:, 1:4, :],
                    rcp[:].unsqueeze(1).broadcast_to([128, 3, GW]),
                    AluOpType.mult,
                )

                src = col[:].unsqueeze(3).broadcast_to([128, 3, GW, PS])
                dst = exfull[:, h, :, :].rearrange("p c (a b) -> p c a b", b=PS)
                nc.vector.tensor_copy(dst, src)

            ydst = y[:].rearrange("(h p) c w -> p h (c w)", h=2)
            nc.sync.dma_start(ydst, exfull[:].rearrange("p h c w -> p h (c w)"))

    nc.finalize()
    return nc


def _make_weights():
    wm = np.zeros((KUV, 2 * 128), dtype=np.float32)
    for k in range(KPC):
        h, rem = divmod(k, 4)
        wm[k * KS * KS:(k + 1) * KS * KS,
           h * 128 + rem * PS:h * 128 + (rem + 1) * PS] = 1.0
    return wm


def _pack_inputs(rgb):
    """rgb [1,3,H,W] -> list of per-core packed x arrays [72, 448]."""
    rgb0 = np.asarray(rgb)[0]  # [3,H,W]
    wm = _make_weights()
    cols = (np.arange(GW)[:, None] * PS + np.arange(KS)[None, :] - 1)  # [64,3]
    colvalid = cols >= 0
    cols_c = np.clip(cols, 0, W - 1)
    packed = []
    for m in range(NCORES):
        rows = (
            (np.arange(KPC)[:, None] + KPC * m) * PS
            + np.arange(KS)[None, :] - 1
        )  # [8,3]
        rowvalid = rows >= 0
        rows_c = np.clip(rows, 0, H - 1)
        Xc = rgb0[:, rows_c.reshape(-1), :][:, :, cols_c.reshape(-1)]
        X = Xc.reshape(3, KPC, KS, GW, KS).copy()
        X[:, ~rowvalid, :, :] = FILL
        X[:, :, :, ~colvalid.reshape(GW, KS)] = FILL
        # -> [(k,u,v), (c,j)]
        Xp = X.transpose(1, 2, 4, 0, 3).reshape(KUV, 3 * GW)
        packed.append(np.ascontiguousarray(np.hstack([Xp, wm])))
    return packed


_NC_CACHE = {}


def _get_nc():
    if "nc" not in _NC_CACHE:
        _NC_CACHE["nc"] = _build_nc()
    return _NC_CACHE["nc"]


def kernel(rgb, param_num_bins, param_kernel_size, param_pixel_size, **_ignored):
    assert int(param_num_bins) == NB
    assert int(param_kernel_size) == KS
    assert int(param_pixel_size) == PS
    nc = _get_nc()
    xs = _pack_inputs(rgb)
    in_maps = [{"x": xs[m]} for m in range(NCORES)]
    res = run_bass_kernel_spmd(nc, in_maps, core_ids=list(range(NCORES)))
    out = np.empty((1, 3, H, W), dtype=np.float32)
    for m in range(NCORES):
        out[0, :, m * ROWS_PC:(m + 1) * ROWS_PC, :] = (
            res.results[m]["y"].transpose(1, 0, 2)
        )
    return out
